# revision 19
# baseline (speedup 1.0000x reference)
import os
import sys
from contextlib import ExitStack

for _p in ("/opt/trn_rl_repo", "/root/.axon_site/_ro/trn_rl_repo"):
    if os.path.isdir(_p) and _p not in sys.path:
        sys.path.insert(0, _p)

import numpy as np
import ml_dtypes

import concourse.bass as bass
import concourse.mybir as mybir
import concourse.tile as tile
from concourse import bacc

f32 = mybir.dt.float32
bf16 = mybir.dt.bfloat16
OP = mybir.AluOpType
AF = mybir.ActivationFunctionType

B, S, D = 2, 2048, 1024
H, DK = 16, 64
HI, DI = 4, 64
K = 512
NCORES = 8
OWN = 512
NT = 4
NK = NT - 1
NITER = 24
NEG = -1e30
MASKVAL = 30.0
WD = [512 * (k + 1) for k in range(NT)]

_GRAPH = None
DEBUG = False


def _build_graph():
    nc = bacc.Bacc("TRN2", target_bir_lowering=False, debug=False,
                   num_devices=NCORES)

    dp = {}
    def din(name, shape, dt):
        dp[name] = nc.dram_tensor(name, shape, dt, kind="ExternalInput").ap()
    din("xT", [8, 128, S], f32)
    din("xTown", [8, 128, OWN], f32)
    din("xTbf", [8, 128, S], bf16)
    din("xTownbf", [8, 128, OWN], bf16)
    din("wqkT", [16, 128, 8, 128], bf16)
    din("wvT", [8, 128, D], bf16)
    din("wqiT", [128, 8, HI * DI], f32)
    din("wkiT", [128, 8, DI], f32)
    din("wwiT", [128, 8, HI], f32)
    din("woE", [8, 16, 64, 128], bf16)
    din("causal", [NK, 128, 512], f32)
    din("ident", [128, 128], f32)
    din("S2T", [128, 128], bf16)
    din("ropeQc", [128, OWN], bf16)
    din("ropeQs", [128, OWN], bf16)
    din("ropeKc", [128, S], bf16)
    din("ropeKs", [128, S], bf16)
    din("iota8", [128, 8], f32)
    din("ones64", [1, 64], f32)
    dp["yT"] = nc.dram_tensor("yT", [D, OWN], f32, kind="ExternalOutput").ap()
    if DEBUG:
        for k in range(1, NT):
            dp[f"dbg_r{k}"] = nc.dram_tensor(f"dbg_r{k}", [128, WD[k]], f32,
                                             kind="ExternalOutput").ap()
            dp[f"dbg_m{k}"] = nc.dram_tensor(f"dbg_m{k}", [128, WD[k]], f32,
                                             kind="ExternalOutput").ap()
        dp["dbg_st"] = nc.dram_tensor("dbg_st", [128, 8 * NK], f32,
                                      kind="ExternalOutput").ap()
        dp["dbg_pt0"] = nc.dram_tensor("dbg_pt0", [16, 128, 512], f32,
                                       kind="ExternalOutput").ap()
        dp["dbg_otd0"] = nc.dram_tensor("dbg_otd0", [65, 512], f32,
                                        kind="ExternalOutput").ap()
        dp["dbg_q0"] = nc.dram_tensor("dbg_q0", [128, OWN], f32,
                                      kind="ExternalOutput").ap()
        dp["dbg_k0"] = nc.dram_tensor("dbg_k0", [128, S], f32,
                                      kind="ExternalOutput").ap()

    with tile.TileContext(nc) as tc:
        _emit(nc, tc, dp)

    nc.compile()
    return nc


def _emit(nc, tc, dp):
    ctx = ExitStack()
    ctxA = ExitStack()
    ctxB = ExitStack()
    ctxC = ExitStack()
    ctxD = ExitStack()
    ctxAt = ExitStack()

    const = ctx.enter_context(tc.tile_pool(name="const", bufs=1))
    st_p = ctx.enter_context(tc.tile_pool(name="state", bufs=1))
    y_p = ctx.enter_context(tc.tile_pool(name="y", bufs=2))
    qkv_p = ctx.enter_context(tc.tile_pool(name="qkv", bufs=1))
    maskT_p = ctx.enter_context(tc.tile_pool(name="maskT", bufs=1))
    vsb_p = ctx.enter_context(tc.tile_pool(name="vsb", bufs=1))
    mm_ps = ctx.enter_context(tc.tile_pool(name="mmps", bufs=3, space="PSUM"))

    ident = const.tile([128, 128], f32)
    nc.sync.dma_start(ident[:], dp["ident"])
    iota8 = const.tile([128, 8], f32)
    nc.sync.dma_start(iota8[:], dp["iota8"])
    ones64 = const.tile([1, 64], f32)
    nc.sync.dma_start(ones64[:], dp["ones64"])
    s2t = const.tile([128, 128], bf16)
    nc.sync.dma_start(s2t[:], dp["S2T"])
    ropeQc = const.tile([128, OWN], bf16)
    nc.sync.dma_start(ropeQc[:], dp["ropeQc"])
    ropeQs = const.tile([128, OWN], bf16)
    nc.sync.dma_start(ropeQs[:], dp["ropeQs"])
    ropeKc = const.tile([128, S], bf16)
    nc.sync.dma_start(ropeKc[:], dp["ropeKc"])
    ropeKs = const.tile([128, S], bf16)
    nc.sync.dma_start(ropeKs[:], dp["ropeKs"])

    idx_p = ctxA.enter_context(tc.tile_pool(name="idxo", bufs=1))
    relu_p = ctxA.enter_context(tc.tile_pool(name="relu", bufs=2))
    xT_p = ctxB.enter_context(tc.tile_pool(name="xTp", bufs=2))
    widx_p = ctxB.enter_context(tc.tile_pool(name="widx", bufs=1))
    ki_psp = ctxB.enter_context(tc.tile_pool(name="kips", bufs=1, space="PSUM"))

    xTown = [xT_p.tile([128, OWN], f32, name=f"xTo{c}") for c in range(8)]
    for c in range(8):
        nc.sync.dma_start(xTown[c][:], dp["xTown"][c])

    wqi = widx_p.tile([128, 8 * HI * DI], f32)
    nc.sync.dma_start(wqi[:], dp["wqiT"].rearrange("p c w -> p (c w)"))
    wki = widx_p.tile([128, 8 * DI], f32)
    nc.sync.dma_start(wki[:], dp["wkiT"].rearrange("p c w -> p (c w)"))
    wwi = widx_p.tile([128, 8 * HI], f32)
    nc.sync.dma_start(wwi[:], dp["wwiT"].rearrange("p c w -> p (c w)"))

    qiT = []
    for h in range(HI):
        ps = mm_ps.tile([64, OWN], f32, tag="mm", name="mm")
        for c in range(8):
            nc.tensor.matmul(ps[:], wqi[:, c * 256 + h * 64:c * 256 + (h + 1) * 64],
                             xTown[c][:], start=(c == 0), stop=(c == 7))
        t = idx_p.tile([64, OWN], f32, name=f"qiT{h}")
        nc.vector.tensor_copy(t[:], ps[:])
        qiT.append(t)

    kiT = idx_p.tile([64, S], f32)
    ki_ps = [ki_psp.tile([64, 512], f32, name=f"kips{j}") for j in range(4)]
    for c in range(8):
        xts = xT_p.tile([128, S], f32, tag="xts", name="xts")
        nc.sync.dma_start(xts[:], dp["xT"][c])
        for j in range(4):
            nc.tensor.matmul(ki_ps[j][:], wki[:, c * 64:(c + 1) * 64],
                             xts[:, j * 512:(j + 1) * 512],
                             start=(c == 0), stop=(c == 7))
    for j in range(4):
        nc.vector.tensor_copy(kiT[:, j * 512:(j + 1) * 512], ki_ps[j][:])

    wiown = []
    for k in range(NT):
        ps = mm_ps.tile([128, HI], f32, tag="mm", name="mm")
        for c in range(8):
            nc.tensor.matmul(ps[:], xTown[c][:, k * 128:(k + 1) * 128],
                             wwi[:, c * HI:(c + 1) * HI],
                             start=(c == 0), stop=(c == 7))
        t = idx_p.tile([128, HI], f32, name=f"wio{k}")
        nc.vector.tensor_copy(t[:], ps[:])
        wiown.append(t)

    ctxB.close()

    r_p = ctxC.enter_context(tc.tile_pool(name="rp", bufs=1))
    scr_p = ctxC.enter_context(tc.tile_pool(name="scr", bufs=1))
    cm_p = ctxC.enter_context(tc.tile_pool(name="cm", bufs=1))
    tr_ps = ctxC.enter_context(tc.tile_pool(name="trps", bufs=2, space="PSUM"))
    causal = [r_p.tile([128, 512], f32, name=f"causal{k}") for k in range(NK)]
    for k in range(NK):
        nc.sync.dma_start(causal[k][:], dp["causal"][k])

    r_t = [None] + [r_p.tile([128, WD[k]], f32, name=f"r{k}") for k in range(1, NT)]
    for k in range(1, NT):
        for j in range(k + 1):
            for h in range(HI):
                ps = mm_ps.tile([128, 512], f32, tag="mm", name="mm")
                nc.tensor.matmul(ps[:], qiT[h][:, k * 128:(k + 1) * 128],
                                 kiT[:, j * 512:(j + 1) * 512],
                                 start=True, stop=True)
                rl = relu_p.tile([128, 512], f32, tag="relu", name="relu")
                nc.scalar.activation(rl[:], ps[:], AF.Relu)
                dst = r_t[k][:, j * 512:(j + 1) * 512]
                if h == 0:
                    nc.vector.tensor_scalar(dst, rl[:], wiown[k][:, 0:1], None,
                                            op0=OP.mult)
                else:
                    nc.vector.scalar_tensor_tensor(dst, rl[:], wiown[k][:, h:h + 1],
                                                   dst, op0=OP.mult, op1=OP.add)
        nc.vector.tensor_tensor(r_t[k][:, k * 512:(k + 1) * 512],
                                r_t[k][:, k * 512:(k + 1) * 512],
                                causal[k - 1][:], op=OP.add)
        if DEBUG:
            nc.sync.dma_start(dp[f"dbg_r{k}"], r_t[k][:])
    lo = st_p.tile([128, NK], f32)
    hi = st_p.tile([128, NK], f32)
    chi = st_p.tile([128, NK], f32)
    cnt = st_p.tile([128, NK], f32)
    mid = st_p.tile([128, NK], f32)
    ge = st_p.tile([128, NK], f32)
    d1 = st_p.tile([128, NK], f32)
    t1 = st_p.tile([128, NK], f32)
    m1 = st_p.tile([128, NK], f32)
    Pst = st_p.tile([128, NK], f32)
    Zc = st_p.tile([128, NK], f32)
    jj = st_p.tile([128, NK], f32)
    theta = st_p.tile([128, NK], f32)
    kmp = st_p.tile([128, NK], f32)
    m8 = st_p.tile([128, 8], f32)
    oh8 = st_p.tile([128, 8], f32)
    za = st_p.tile([128, NK], f32)
    zb = st_p.tile([128, NK], f32)

    nc.vector.memset(lo[:], -64.0)
    nc.vector.memset(hi[:], 64.0)
    nc.vector.memset(chi[:], 0.0)
    nc.vector.memset(mid[:], 0.0)

    cmask = [None] + [cm_p.tile([128, WD[k]], bf16, name=f"cm{k}")
                      for k in range(1, NT)]
    for it in range(NITER + 1):
        for k in range(1, NT):
            eng = nc.vector
            eng.tensor_scalar(cmask[k][:], r_t[k][:], mid[:, k - 1:k], None,
                              op0=OP.is_gt, op1=OP.add,
                              accum_out=cnt[:, k - 1:k])
        if it == 0:
            nc.vector.tensor_copy(Pst[:], cnt[:])
        nc.vector.tensor_scalar(ge[:], cnt[:], 512.0, None, op0=OP.is_ge)
        nc.vector.tensor_tensor(d1[:], mid[:], lo[:], op=OP.subtract)
        nc.vector.tensor_tensor(t1[:], ge[:], d1[:], op=OP.mult)
        nc.vector.tensor_tensor(lo[:], lo[:], t1[:], op=OP.add)
        nc.vector.tensor_tensor(d1[:], hi[:], mid[:], op=OP.subtract)
        nc.vector.tensor_tensor(t1[:], ge[:], d1[:], op=OP.mult)
        nc.vector.tensor_tensor(hi[:], mid[:], t1[:], op=OP.add)
        nc.vector.tensor_tensor(d1[:], chi[:], cnt[:], op=OP.subtract)
        nc.vector.tensor_tensor(t1[:], ge[:], d1[:], op=OP.mult)
        nc.vector.tensor_tensor(chi[:], cnt[:], t1[:], op=OP.add)
        nc.vector.tensor_tensor(m1[:], lo[:], hi[:], op=OP.add)
        nc.vector.tensor_scalar(mid[:], m1[:], 0.5, None, op0=OP.mult)

    nc.vector.tensor_scalar(jj[:], chi[:], -1.0, 512.0, op0=OP.mult, op1=OP.add)

    for k in range(1, NT):
        scrA = scr_p.tile([128, S], f32, tag="scrA", name="scrA")
        scrB = scr_p.tile([128, S], bf16, tag="scrB", name="scrB")
        W = WD[k]
        zt_ = cmask[k]
        nc.vector.tensor_scalar(scrA[:, :W], r_t[k][:], hi[:, k - 1:k], NEG,
                                op0=OP.is_gt, op1=OP.mult)
        nc.vector.tensor_tensor(scrA[:, :W], r_t[k][:], scrA[:, :W], op=OP.add)
        nc.vector.max(out=m8[:], in_=scrA[:, :W])
        nc.vector.tensor_scalar(oh8[:], iota8[:], jj[:, k - 1:k], None,
                                op0=OP.is_equal)
        nc.vector.tensor_tensor(oh8[:], oh8[:], m8[:], op=OP.mult)
        nc.vector.tensor_reduce(theta[:, k - 1:k], oh8[:],
                                axis=mybir.AxisListType.X, op=OP.add)
        nc.vector.tensor_scalar(zt_[:], r_t[k][:], 0.0, None,
                                op0=OP.is_equal, op1=OP.add,
                                accum_out=Zc[:, k - 1:k])
        nc.vector.tensor_tensor_scan(scrB[:, :W], zt_[:], zt_[:], 0.0,
                                     op0=OP.add, op1=OP.bypass)
        nc.vector.tensor_scalar(za[:, k - 1:k], Pst[:, k - 1:k], 512.0, None,
                                op0=OP.is_lt)
        nc.vector.tensor_tensor(zb[:, k - 1:k], Pst[:, k - 1:k], Zc[:, k - 1:k],
                                op=OP.add)
        nc.vector.tensor_scalar(zb[:, k - 1:k], zb[:, k - 1:k], 512.0, None,
                                op0=OP.is_ge)
        nc.vector.tensor_tensor(za[:, k - 1:k], za[:, k - 1:k], zb[:, k - 1:k],
                                op=OP.mult)
        nc.vector.tensor_scalar(zb[:, k - 1:k], Pst[:, k - 1:k], -1.0, 513.0,
                                op0=OP.mult, op1=OP.add)
        nc.vector.tensor_tensor(zb[:, k - 1:k], zb[:, k - 1:k], za[:, k - 1:k],
                                op=OP.mult)
        nc.vector.tensor_scalar(kmp[:, k - 1:k], zb[:, k - 1:k], 1.0, None,
                                op0=OP.subtract)
        nc.vector.scalar_tensor_tensor(scrB[:, :W], scrB[:, :W], kmp[:, k - 1:k],
                                       zt_[:], op0=OP.is_le, op1=OP.mult)
        nc.vector.scalar_tensor_tensor(zt_[:], r_t[k][:], theta[:, k - 1:k],
                                       scrB[:, :W], op0=OP.is_gt, op1=OP.max)
        nc.vector.tensor_scalar(r_t[k][:], zt_[:], 1.0, MASKVAL,
                                op0=OP.subtract, op1=OP.mult)

    if DEBUG:
        for k in range(1, NT):
            nc.sync.dma_start(dp[f"dbg_m{k}"], r_t[k][:])
        for i, tt_ in enumerate([theta, chi, Pst, Zc, kmp, jj, lo, hi]):
            nc.sync.dma_start(dp["dbg_st"][:, i * NK:(i + 1) * NK], tt_[:])

    maskT = [maskT_p.tile([128, 512], bf16, name=f"mT{m}") for m in range(16)]
    for m in range(4):
        nc.vector.memset(maskT[m][:, 0:128], 0.0)
    for k in range(1, NT):
        for m in range(4 * (k + 1)):
            tp = tr_ps.tile([128, 128], f32, tag="tr", name="tr")
            nc.tensor.transpose(tp[:], r_t[k][:, m * 128:(m + 1) * 128], ident[:])
            nc.vector.tensor_copy(maskT[m][:, k * 128:(k + 1) * 128], tp[:])

    ctxC.close()
    ctxA.close()

    xbf_p = ctxD.enter_context(tc.tile_pool(name="xbf", bufs=1))
    wqk_p = ctxD.enter_context(tc.tile_pool(name="wqk", bufs=2))
    xbf = [xbf_p.tile([128, S], bf16, name=f"xb{c}") for c in range(8)]
    xobf = [xbf_p.tile([128, OWN], bf16, name=f"xob{c}") for c in range(8)]
    for c in range(8):
        nc.sync.dma_start(xbf[c][:], dp["xTbf"][c])
        nc.sync.dma_start(xobf[c][:], dp["xTownbf"][c])

    qT = []
    for p in range(8):
        wt = wqk_p.tile([128, 1024], bf16, tag="wq", name="wq")
        nc.sync.dma_start(wt[:], dp["wqkT"][p].rearrange("p c w -> p (c w)"))
        ps = mm_ps.tile([128, OWN], f32, tag="mm", name="mm")
        for c in range(8):
            nc.tensor.matmul(ps[:], wt[:, c * 128:(c + 1) * 128], xobf[c][:],
                             start=(c == 0), stop=(c == 7))
        raw = wqk_p.tile([128, OWN], bf16, tag="qraw", name="qraw")
        nc.vector.tensor_copy(raw[:], ps[:])
        ps2 = mm_ps.tile([128, OWN], f32, tag="mm", name="mm")
        nc.tensor.matmul(ps2[:], s2t[:], raw[:], start=True, stop=True)
        t = qkv_p.tile([128, OWN], bf16, name=f"qT{p}")
        nc.vector.tensor_tensor(t[:], raw[:], ropeQc[:], op=OP.mult)
        sw = wqk_p.tile([128, OWN], bf16, tag="qsw", name="qsw")
        nc.vector.tensor_tensor(sw[:], ps2[:], ropeQs[:], op=OP.mult)
        nc.vector.tensor_tensor(t[:], t[:], sw[:], op=OP.add)
        if DEBUG and p == 0:
            nc.gpsimd.dma_start(dp["dbg_q0"], t[:])
        qT.append(t)

    kT = []
    for p in range(8):
        wt = wqk_p.tile([128, 1024], bf16, tag="wk", name="wk")
        nc.sync.dma_start(wt[:], dp["wqkT"][8 + p].rearrange("p c w -> p (c w)"))
        t = qkv_p.tile([128, S], bf16, name=f"kT{p}")
        for j in range(4):
            ps = mm_ps.tile([128, 512], f32, tag="mm", name="mm")
            for c in range(8):
                nc.tensor.matmul(ps[:], wt[:, c * 128:(c + 1) * 128],
                                 xbf[c][:, j * 512:(j + 1) * 512],
                                 start=(c == 0), stop=(c == 7))
            raw = wqk_p.tile([128, 512], bf16, tag="kraw", name="kraw")
            nc.vector.tensor_copy(raw[:], ps[:])
            ps2 = mm_ps.tile([128, 512], f32, tag="mm", name="mm")
            nc.tensor.matmul(ps2[:], s2t[:], raw[:], start=True, stop=True)
            dst = t[:, j * 512:(j + 1) * 512]
            nc.vector.tensor_tensor(dst, raw[:],
                                    ropeKc[:, j * 512:(j + 1) * 512], op=OP.mult)
            sw = wqk_p.tile([128, 512], bf16, tag="ksw", name="ksw")
            nc.vector.tensor_tensor(sw[:], ps2[:],
                                    ropeKs[:, j * 512:(j + 1) * 512], op=OP.mult)
            nc.vector.tensor_tensor(dst, dst, sw[:], op=OP.add)
            if DEBUG and p == 0:
                nc.gpsimd.dma_start(dp["dbg_k0"][:, j * 512:(j + 1) * 512], dst)
        kT.append(t)

    wv_p = ctxD.enter_context(tc.tile_pool(name="wv", bufs=1))
    wvt = [wv_p.tile([128, D], bf16, name=f"wv{c}") for c in range(8)]
    for c in range(8):
        nc.sync.dma_start(wvt[c][:], dp["wvT"][c])
    vsb = [vsb_p.tile([128, H * 65], bf16, name=f"v{m}") for m in range(16)]
    for m in range(16):
        for half in range(2):
            ps = mm_ps.tile([128, 512], f32, tag="mm", name="mm")
            for c in range(8):
                nc.tensor.matmul(ps[:], xbf[c][:, m * 128:(m + 1) * 128],
                                 wvt[c][:, half * 512:(half + 1) * 512],
                                 start=(c == 0), stop=(c == 7))
            dst3 = vsb[m][:, half * 8 * 65:(half + 1) * 8 * 65] \
                .rearrange("p (h d) -> p h d", d=65)[:, :, 0:64]
            src3 = ps[:].rearrange("p (h d) -> p h d", d=64)
            nc.vector.tensor_copy(dst3, src3)
        nc.vector.memset(vsb[m][:, 64::65], 1.0)
    ctxD.close()

    pt_p = ctxAt.enter_context(tc.tile_pool(name="pT", bufs=20))
    sc_p = ctxAt.enter_context(tc.tile_pool(name="scomb", bufs=3))
    ot_p = ctxAt.enter_context(tc.tile_pool(name="ot", bufs=3))
    woe_p = ctxAt.enter_context(tc.tile_pool(name="woe", bufs=2))
    pv_ps = ctxAt.enter_context(tc.tile_pool(name="pvps", bufs=2, space="PSUM"))

    SCALE = float(1.0 / np.sqrt(np.float32(DK)))
    otbf_all = []
    for h in range(H):
        p_, half = h // 2, h % 2
        base = half * 64
        pts = []
        for m in range(16):
            kmin = m // 4
            sps = mm_ps.tile([128, 512], f32, tag="mm", name="mm")
            for k in range(kmin, NT):
                nc.tensor.matmul(sps[:, k * 128:(k + 1) * 128],
                                 kT[p_][base:base + 64, m * 128:(m + 1) * 128],
                                 qT[p_][base:base + 64, k * 128:(k + 1) * 128],
                                 start=True, stop=True)
            pt = pt_p.tile([128, 512], bf16, tag="pt", name="pt")
            if kmin == 0:
                nc.scalar.activation(pt[:, 0:128], sps[:, 0:128], AF.Exp,
                                     scale=SCALE)
                kmin_m = 1
            else:
                kmin_m = kmin
            sc = sc_p.tile([128, 512], f32, tag="sc", name="sc")
            w = 512 - kmin_m * 128
            nc.vector.scalar_tensor_tensor(sc[:, :w], sps[:, kmin_m * 128:],
                                           SCALE, maskT[m][:, kmin_m * 128:],
                                           op0=OP.mult, op1=OP.add)
            nc.scalar.activation(pt[:, kmin_m * 128:], sc[:, :w], AF.Exp)
            if DEBUG and h == 0:
                dbf = sc_p.tile([128, 512], f32, tag="dbf", name="dbf")
                nc.vector.memset(dbf[:], 0.0)
                nc.vector.tensor_copy(dbf[:, kmin * 128:], pt[:, kmin * 128:])
                nc.sync.dma_start(dp["dbg_pt0"][m], dbf[:])
            pts.append(pt)

        otd = ot_p.tile([65, 512], f32, tag="otd", name="otd")
        for k in range(NT):
            ps = pv_ps.tile([65, 128], f32, tag="pv", name="pv")
            for m in range(4 * (k + 1)):
                nc.tensor.matmul(ps[:], vsb[m][:, h * 65:(h + 1) * 65],
                                 pts[m][:, k * 128:(k + 1) * 128],
                                 start=(m == 0), stop=(m == 4 * k + 3))
            nc.vector.tensor_copy(otd[:, k * 128:(k + 1) * 128], ps[:])

        if DEBUG and h == 0:
            nc.sync.dma_start(dp["dbg_otd0"], otd[:])
        rden = ot_p.tile([1, 512], f32, tag="rden", name="rden")
        nc.vector.reciprocal(rden[:], otd[64:65, :])
        bc = mm_ps.tile([64, 512], f32, tag="mm", name="mm")
        nc.tensor.matmul(bc[:], ones64[:], rden[:], start=True, stop=True)
        otbf = ot_p.tile([64, 512], bf16, name=f"otbf{h}")
        nc.vector.tensor_tensor(otbf[:], otd[0:64, :], bc[:], op=OP.mult)
        otbf_all.append(otbf)

    for e in range(8):
        woe = woe_p.tile([64, 16 * 128], bf16, tag="woe", name="woe")
        nc.sync.dma_start(woe[:].rearrange("p (h w) -> p h w", w=128),
                          dp["woE"][e].rearrange("h p w -> p h w"))
        ps = mm_ps.tile([128, 512], f32, tag="mm", name="mm")
        for h in range(H):
            nc.tensor.matmul(ps[:], woe[:, h * 128:(h + 1) * 128],
                             otbf_all[h][:], start=(h == 0), stop=(h == 15))
        yt = y_p.tile([128, 512], f32, tag="yt", name="yt")
        nc.vector.tensor_copy(yt[:], ps[:])
        nc.sync.dma_start(dp["yT"][e * 128:(e + 1) * 128, :], yt[:])

    ctxAt.close()
    ctx.close()


def _host_prep(x, w_qkv, w_o, w_qi, w_ki, w_wi):
    xf = np.ascontiguousarray(x, np.float32)
    bf = ml_dtypes.bfloat16
    wqkvT = np.ascontiguousarray(np.asarray(w_qkv, np.float32).T)
    wqk_blk = np.ascontiguousarray(
        wqkvT[:, :2 * D].reshape(8, 128, 16, 128).transpose(2, 1, 0, 3)).astype(bf)
    wv_blk = np.ascontiguousarray(
        wqkvT[:, 2 * D:].reshape(8, 128, D)).astype(bf)
    wqi_blk = np.ascontiguousarray(
        np.asarray(w_qi, np.float32).T.reshape(8, 128, HI * DI).transpose(1, 0, 2))
    wki_blk = np.ascontiguousarray(
        np.asarray(w_ki, np.float32).T.reshape(8, 128, DI).transpose(1, 0, 2))
    wwi_blk = np.ascontiguousarray(
        np.asarray(w_wi, np.float32).T.reshape(8, 128, HI).transpose(1, 0, 2))
    woE_blk = np.ascontiguousarray(
        np.asarray(w_o, np.float32).T.reshape(16, 64, 8, 128)
        .transpose(2, 0, 1, 3)).astype(bf)
    ident = np.eye(128, dtype=np.float32)
    iota8 = np.tile(np.arange(8, dtype=np.float32), (128, 1))
    ones64 = np.ones((1, 64), np.float32)
    S64 = np.zeros((64, 64), np.float32)
    for i in range(32):
        S64[2 * i, 2 * i + 1] = -1.0
        S64[2 * i + 1, 2 * i] = 1.0
    S2 = np.zeros((128, 128), np.float32)
    S2[:64, :64] = S64
    S2[64:, 64:] = S64
    S2T = np.ascontiguousarray(S2.T).astype(bf)
    inv = (1.0 / 10000.0 ** (np.arange(0, 64, 2, dtype=np.float32) / 64.0))
    pos_all = np.arange(S, dtype=np.float32)
    ang_all = pos_all[:, None] * inv[None, :]
    cos_all = np.cos(ang_all); sin_all = np.sin(ang_all)
    def rope_tile(pos_idx):
        cosd = np.repeat(cos_all[pos_idx], 2, axis=1).T
        sind = np.repeat(sin_all[pos_idx], 2, axis=1).T
        return (np.concatenate([cosd, cosd], 0).astype(bf),
                np.concatenate([sind, sind], 0).astype(bf))
    ropeKc_t, ropeKs_t = rope_tile(np.arange(S))

    in_maps = []
    for c in range(NCORES):
        b, q = c // 4, c % 4
        xT = np.ascontiguousarray(xf[b].T).reshape(8, 128, S)
        xTown = np.ascontiguousarray(xf[b, q::4].T).reshape(8, 128, OWN)
        causal = np.zeros((NK, 128, 512), np.float32)
        for k in range(1, NT):
            t_rows = q + 4 * (128 * k + np.arange(128))
            s_cols = 512 * k + np.arange(512)
            causal[k - 1] = np.where(s_cols[None, :] <= t_rows[:, None],
                                     0.0, NEG).astype(np.float32)
        ropeQc_t, ropeQs_t = rope_tile(np.arange(q, S, 4))
        in_maps.append({
            "xT": xT, "xTown": xTown,
            "xTbf": xT.astype(bf), "xTownbf": xTown.astype(bf),
            "wqkT": wqk_blk, "wvT": wv_blk, "wqiT": wqi_blk, "wkiT": wki_blk,
            "wwiT": wwi_blk, "woE": woE_blk,
            "causal": causal, "ident": ident, "iota8": iota8, "ones64": ones64,
            "S2T": S2T, "ropeQc": ropeQc_t, "ropeQs": ropeQs_t,
            "ropeKc": ropeKc_t, "ropeKs": ropeKs_t,
        })
    return in_maps


def get_graph():
    global _GRAPH
    if _GRAPH is None:
        _GRAPH = _build_graph()
    return _GRAPH


def kernel(x, w_qkv, w_o, w_qi, w_ki, w_wi):
    from concourse.bass_utils import run_bass_kernel_spmd
    nc = get_graph()
    in_maps = _host_prep(x, w_qkv, w_o, w_qi, w_ki, w_wi)
    res = run_bass_kernel_spmd(nc, in_maps, core_ids=list(range(NCORES)))
    out = np.empty((B, S, D), np.float32)
    for c in range(NCORES):
        b, q = c // 4, c % 4
        out[b, q::4, :] = np.asarray(res.results[c]["yT"], np.float32).T
    return out
